# revision 1
# baseline (speedup 1.0000x reference)
"""Trainium2 Bass kernel for nn_AtomwiseLinear (histogram_binning).

Pipeline (8 NeuronCores, SPMD, no collectives needed):
  host: shard nodes across cores; partition the 32M edge-endpoint entries by
        owning node bucket (hierarchical sharding: core -> 512-node bucket),
        padded to fixed-size buckets; co-locate the other-endpoint's type bit
        with each entry record; ship x transposed/permuted to the hist layout.
  device (per core):
    A) histogram: per bucket, build 2-level one-hots (lo: 32 bins, hi: 16 bins
       x 2 blocks [plain | type-weighted]) with DVE is_equal, contract with PE
       matmuls accumulating in PSUM -> count[v] and A[v] (# type-1 neighbors).
    B) decode: crit = 3*(count>10) + mix, mix from (count, A, own type).
    C) dense: out^T = mask * ((x @ W) / sqrt(30)) via PE matmul + DVE mask.
  host: inverse-permute/transpose device outputs into [1M, 30].
"""

import os
import sys

sys.path.insert(0, "/opt/trn_rl_repo")

import numpy as np
import ml_dtypes

from concourse import bacc, bass, mybir
import concourse.tile as tile
from concourse.bass_utils import run_bass_kernel_spmd

BF16 = ml_dtypes.bfloat16

NCORES = 8
D = 30
WINDOW = 5
DEG_THRESH = 10

MINI = bool(int(os.environ.get("KMINI", "0")))

if MINI:
    LOG_SHARD = 11          # nodes per core
    TPB = 8                 # tiles (of 128 entries) per bucket
else:
    LOG_SHARD = 17
    TPB = int(os.environ.get("KTPB", "132"))

SHARD = 1 << LOG_SHARD
KUNROLL = bool(int(os.environ.get("KUNROLL", "0")))
LO = 32                     # low-bin count (rhs one-hot width)
HI = 16                     # high-bin count (lhsT block width)
BUCKET = LO * HI            # 512 nodes per bucket
NBUCK = SHARD // BUCKET     # buckets per core
CAP = TPB * 128             # padded entries per bucket
NTD = min(512, NBUCK * HI)  # dense-phase node tile (psum free-dim)
CW = NBUCK * LO             # hist free width (columns per hi-chunk)

F32 = mybir.dt.float32
BF = mybir.dt.bfloat16
U8 = mybir.dt.uint8
KU8 = bool(int(os.environ.get("KU8", "1")))
KFUSE = bool(int(os.environ.get("KFUSE", "1")))
KLOOP2 = min(int(os.environ.get("KLOOP2", "32")), NBUCK)
KCONTIG = bool(int(os.environ.get("KCONTIG", "1")))
OH = mybir.dt.float8e4 if KU8 else BF
IOTA_NPDT = np.uint8 if KU8 else BF16


def _host_prep(x, W, edge_index, atom_types):
    """Shard + bucket inputs. Returns (in_maps, col_perm, n_real)."""
    n = x.shape[0]
    e0 = np.asarray(edge_index[0], dtype=np.int32)
    e1 = np.asarray(edge_index[1], dtype=np.int32)
    t8 = np.asarray(atom_types, dtype=np.uint8)

    # entry stream: each edge contributes (node=src, w=t[dst]) and (node=dst, w=t[src])
    nodes = np.concatenate([e0, e1])
    wbit = np.concatenate([t8[e1], t8[e0]])

    nbuck_total = NCORES * NBUCK
    gb = (nodes >> 9).astype(np.int16)  # global bucket id (BUCKET=512)
    counts = np.bincount(gb, minlength=nbuck_total)
    if counts.max() > CAP:
        raise RuntimeError(f"bucket overflow: {counts.max()} > {CAP}")
    order = np.argsort(gb, kind="stable")
    gbs = gb[order].astype(np.int64)
    sn = nodes[order]
    sw = wbit[order]
    starts = np.zeros(nbuck_total, dtype=np.int64)
    np.cumsum(counts[:-1], out=starts[1:])
    within = np.arange(nodes.shape[0], dtype=np.int64) - starts[gbs]

    streams = np.full((nbuck_total, 128, 3 * TPB), 255, dtype=np.uint8)
    p = within & 127
    t = within >> 7
    flat = streams.reshape(-1)
    base = gbs * (128 * 3 * TPB) + p * (3 * TPB)
    hi_f = ((sn >> 5) & (HI - 1)).astype(np.uint8)
    flat[base + t] = (sn & (LO - 1)).astype(np.uint8)
    flat[base + TPB + t] = hi_f
    flat[base + 2 * TPB + t] = hi_f + 32 - 32 * sw.astype(np.uint8)

    # dense-layout permutation: local node L -> hist column order
    L = np.arange(SHARD, dtype=np.int64)
    col = (L & (LO - 1)) * (NBUCK * HI) + (L >> 9) * HI + ((L >> 5) & (HI - 1))
    inv = np.empty(SHARD, dtype=np.int64)
    inv[col] = L  # node at flat hist position j is inv[j]

    iota_lo = np.ascontiguousarray(np.broadcast_to(
        np.tile(np.arange(LO, dtype=np.float32), TPB), (128, TPB * LO)
    ).astype(IOTA_NPDT))
    iota_hi = np.ascontiguousarray(np.broadcast_to(
        np.tile(np.arange(HI, dtype=np.float32), TPB), (128, TPB * HI)
    ).astype(IOTA_NPDT))
    d5 = (np.arange(D, dtype=np.float32) // WINDOW).reshape(D, 1)
    wmat = np.asarray(W, dtype=np.float32)

    in_maps = []
    n_real = []
    for c in range(NCORES):
        lo_g = c * SHARD
        hi_g = min(n, (c + 1) * SHARD)
        nc_real = max(0, hi_g - lo_g)
        n_real.append(nc_real)
        xs = np.zeros((SHARD, D), dtype=np.float32)
        ts = np.zeros(SHARD, dtype=np.float32)
        if nc_real > 0:
            xs[:nc_real] = x[lo_g:hi_g]
            ts[:nc_real] = t8[lo_g:hi_g]
        xt = np.ascontiguousarray(xs[inv].T)          # [D, SHARD] in hist order
        th = np.ascontiguousarray(ts[inv]).reshape(LO, NBUCK * HI)
        in_maps.append(
            {
                "streams": streams[c * NBUCK : (c + 1) * NBUCK],
                "xt": xt,
                "th": th,
                "wmat": wmat,
                "iota_lo": iota_lo,
                "iota_hi": iota_hi,
                "d5": d5,
            }
        )
    return in_maps, col, n_real


def build_nc():
    nc = bacc.Bacc("TRN2", target_bir_lowering=False, debug=False, num_devices=NCORES)
    streams_d = nc.dram_tensor("streams", [NBUCK, 128, 3 * TPB], U8, kind="ExternalInput")
    xt_d = nc.dram_tensor("xt", [D, SHARD], F32, kind="ExternalInput")
    th_d = nc.dram_tensor("th", [LO, NBUCK * HI], F32, kind="ExternalInput")
    wmat_d = nc.dram_tensor("wmat", [D, D], F32, kind="ExternalInput")
    IDT = U8 if KU8 else BF
    iota_lo_d = nc.dram_tensor("iota_lo", [128, TPB * LO], IDT, kind="ExternalInput")
    iota_hi_d = nc.dram_tensor("iota_hi", [128, TPB * HI], IDT, kind="ExternalInput")
    d5_d = nc.dram_tensor("d5", [D, 1], F32, kind="ExternalInput")
    outt_d = nc.dram_tensor("outt", [D, SHARD], F32, kind="ExternalOutput")
    critd = nc.dram_tensor("crit_bounce", [LO, NBUCK * HI], F32)
    ybounce = nc.dram_tensor("y_bounce", [D, SHARD], F32)

    with tile.TileContext(nc) as tc:
        with tc.tile_pool(name="const", bufs=1) as cpool:
            iota_lo = cpool.tile([128, TPB * LO], IDT)
            iota_hi = cpool.tile([128, TPB * HI], IDT)
            wsc = cpool.tile([D, D], F32)
            d5 = cpool.tile([D, 1], F32)
            th = cpool.tile([LO, NBUCK * HI], F32)
            hist = cpool.tile([LO, NBUCK * 2 * HI], F32)

            nc.sync.dma_start(out=iota_lo[:], in_=iota_lo_d[:])
            nc.sync.dma_start(out=iota_hi[:], in_=iota_hi_d[:])
            nc.sync.dma_start(out=wsc[:], in_=wmat_d[:])
            nc.scalar.mul(out=wsc[:], in_=wsc[:], mul=float(1.0 / np.sqrt(D)))
            nc.sync.dma_start(out=d5[:], in_=d5_d[:])
            nc.sync.dma_start(out=th[:], in_=th_d[:])

            # ---- Phase A: bucketed 2-level one-hot histogram ----
            wpool = tc.alloc_tile_pool(name="work", bufs=int(os.environ.get("KWBUFS", "6")))
            ppool = tc.alloc_tile_pool(name="psum", bufs=4, space="PSUM")
            import contextlib

            def _bucket_iter():
                if KUNROLL:
                    for bb in range(NBUCK):
                        yield bb
                elif KLOOP2 > 1:
                    assert NBUCK % KLOOP2 == 0
                    with tc.For_i(
                        0, NBUCK, KLOOP2,
                        hint_engines=(mybir.EngineType.PE,),
                    ) as bb:
                        for db in range(KLOOP2):
                            yield bb + db
                else:
                    with tc.For_i(
                        0, NBUCK, 1,
                        hint_engines=(mybir.EngineType.PE,),
                        staggered_reset=bool(int(os.environ.get("KSTAG", "0"))),
                    ) as bb:
                        yield bb

            for b in _bucket_iter():
                raw = wpool.tile([128, 3 * TPB], U8, tag="raw")
                nc.sync.dma_start(out=raw[:], in_=streams_d[bass.ds(b, 1), :, :])
                if KU8:
                    lob = raw[:, 0:TPB]
                    hib = raw[:, TPB : 2 * TPB]
                    hi2 = raw[:, 2 * TPB : 3 * TPB]
                else:
                    lobt = wpool.tile([128, TPB], BF, tag="lob")
                    hibt = wpool.tile([128, TPB], BF, tag="hib")
                    hi2t = wpool.tile([128, TPB], BF, tag="hi2")
                    nc.scalar.copy(out=lobt[:], in_=raw[:, 0:TPB])
                    nc.scalar.copy(out=hibt[:], in_=raw[:, TPB : 2 * TPB])
                    nc.scalar.copy(out=hi2t[:], in_=raw[:, 2 * TPB : 3 * TPB])
                    lob, hib, hi2 = lobt[:], hibt[:], hi2t[:]

                ohlo = wpool.tile([128, TPB * LO], OH, tag="ohlo")
                blk = wpool.tile([128, TPB * 2 * HI], OH, tag="blk")
                ohlo3 = ohlo[:].rearrange("p (t n) -> p t n", n=LO)
                ilo3 = iota_lo[:].rearrange("p (t n) -> p t n", n=LO)
                lob3 = lob.to_broadcast([128, TPB, LO])
                nc.vector.tensor_tensor(
                    out=ohlo3, in0=ilo3, in1=lob3, op=mybir.AluOpType.is_equal
                )
                ihi3 = iota_hi[:].rearrange("p (t n) -> p t n", n=HI)
                hib3 = hib.to_broadcast([128, TPB, HI])
                hi23 = hi2.to_broadcast([128, TPB, HI])
                if KCONTIG:
                    # two contiguous half-tensors; the matmul rhs re-interleaves
                    # them per tile via a 3D AP (keeps DVE writes contiguous)
                    blk4 = blk[:].rearrange("p (s t n) -> p s t n", s=2, n=HI)
                    nc.vector.tensor_tensor(
                        out=blk4[:, 0, :, :], in0=ihi3, in1=hib3,
                        op=mybir.AluOpType.is_equal,
                    )
                    nc.vector.tensor_tensor(
                        out=blk4[:, 1, :, :], in0=ihi3, in1=hi23,
                        op=mybir.AluOpType.is_equal,
                    )
                else:
                    blk3 = blk[:].rearrange("p (t m) -> p t m", m=2 * HI)
                    nc.vector.tensor_tensor(
                        out=blk3[:, :, 0:HI], in0=ihi3, in1=hib3,
                        op=mybir.AluOpType.is_equal,
                    )
                    nc.vector.tensor_tensor(
                        out=blk3[:, :, HI : 2 * HI], in0=ihi3, in1=hi23,
                        op=mybir.AluOpType.is_equal,
                    )

                # fused dense chunk(s) for this iteration:
                # y[:, c0:c0+NTD] = (x@W)/sqrt(D), hidden under the DVE one-hots
                cpb = ((SHARD // NTD) // NBUCK) if KFUSE else 0
                for k in range(cpb):
                    xt_t = wpool.tile([D, NTD], F32, tag="xt1")
                    nc.sync.dma_start(
                        out=xt_t[:],
                        in_=xt_d[:, bass.ds(b * (cpb * NTD) + k * NTD, NTD)],
                    )
                    ps2d = ppool.tile([D, NTD], F32, tag="ps2d")
                    nc.tensor.matmul(ps2d[:], lhsT=wsc[:], rhs=xt_t[:], start=True, stop=True)
                    yt = wpool.tile([D, NTD], F32, tag="yt1")
                    nc.scalar.copy(out=yt[:], in_=ps2d[:])
                    nc.sync.dma_start(
                        out=ybounce[:, bass.ds(b * (cpb * NTD) + k * NTD, NTD)], in_=yt[:]
                    )

                ps = ppool.tile([LO, 2 * HI], F32, tag="ps")
                blk4m = blk[:].rearrange("p (s t n) -> p s t n", s=2, n=HI)
                for t in range(TPB):
                    rhs_t = (
                        blk4m[:, :, t, :] if KCONTIG
                        else blk[:, t * 2 * HI : (t + 1) * 2 * HI]
                    )
                    nc.tensor.matmul(
                        ps[:],
                        lhsT=ohlo[:, t * LO : (t + 1) * LO],
                        rhs=rhs_t,
                        start=(t == 0),
                        stop=(t == TPB - 1),
                    )
                nc.scalar.copy(out=hist[:, bass.ds(b * 2 * HI, 2 * HI)], in_=ps[:])

            wpool.release()
            ppool.release()

            # ---- Phase B: decode crit = 3*(count>10) + mix ----
            hist3 = hist[:].rearrange("p (b u) -> p b u", u=2 * HI)
            cnt = hist3[:, :, 0:HI]
            aa = hist3[:, :, HI : 2 * HI]
            BW = NBUCK * HI
            ta = cpool.tile([LO, BW], F32)
            tb = cpool.tile([LO, BW], F32)
            crit = cpool.tile([LO, BW], F32)
            AL = mybir.AluOpType

            def v3(t):
                return t[:].rearrange("p (b u) -> p b u", u=HI)

            nc.vector.tensor_scalar(out=v3(ta), in0=aa, scalar1=0.0, scalar2=None, op0=AL.is_equal)
            nc.vector.tensor_scalar(out=tb[:], in0=th[:], scalar1=-1.0, scalar2=1.0, op0=AL.mult, op1=AL.add)
            nc.vector.tensor_tensor(out=ta[:], in0=ta[:], in1=tb[:], op=AL.mult)
            nc.vector.tensor_tensor(out=v3(tb), in0=aa, in1=cnt, op=AL.is_equal)
            nc.vector.scalar_tensor_tensor(out=tb[:], in0=tb[:], scalar=2.0, in1=th[:], op0=AL.mult, op1=AL.mult)
            nc.vector.tensor_tensor(out=ta[:], in0=ta[:], in1=tb[:], op=AL.add)
            nc.vector.tensor_scalar(out=v3(tb), in0=cnt, scalar1=0.0, scalar2=None, op0=AL.is_gt)
            nc.vector.tensor_tensor(out=ta[:], in0=ta[:], in1=tb[:], op=AL.mult)
            nc.vector.tensor_scalar(out=v3(tb), in0=cnt, scalar1=float(DEG_THRESH) + 0.5, scalar2=None, op0=AL.is_gt)
            nc.vector.scalar_tensor_tensor(out=crit[:], in0=tb[:], scalar=3.0, in1=ta[:], op0=AL.mult, op1=AL.add)

            # ---- Phase C: dense (x @ W) * scale * mask ----
            dpool = tc.alloc_tile_pool(name="dense", bufs=3)
            dppool = tc.alloc_tile_pool(name="dpsum", bufs=2, space="PSUM") if not KFUSE else None
            nc.sync.dma_start(out=critd[:], in_=crit[:])
            CWD = NBUCK * HI
            for h in range(LO):
                critb = dpool.tile([D, CWD], F32, tag="critb")
                nc.sync.dma_start(out=critb[:], in_=critd[h : h + 1, :].to_broadcast([D, CWD]))
                for j in range(CWD // NTD):
                    c0 = h * CWD + j * NTD
                    yt2 = dpool.tile([D, NTD], F32, tag="yt2")
                    if KFUSE:
                        nc.sync.dma_start(out=yt2[:], in_=ybounce[:, c0 : c0 + NTD])
                    else:
                        xt_t = dpool.tile([D, NTD], F32, tag="xt2")
                        nc.sync.dma_start(out=xt_t[:], in_=xt_d[:, c0 : c0 + NTD])
                        ps2 = dppool.tile([D, NTD], F32, tag="ps2")
                        nc.tensor.matmul(ps2[:], lhsT=wsc[:], rhs=xt_t[:], start=True, stop=True)
                        nc.scalar.copy(out=yt2[:], in_=ps2[:])
                    mt = dpool.tile([D, NTD], F32, tag="mt")
                    nc.vector.scalar_tensor_tensor(
                        out=mt[:],
                        in0=critb[:, j * NTD : (j + 1) * NTD],
                        scalar=d5[:],
                        in1=yt2[:],
                        op0=AL.is_equal,
                        op1=AL.mult,
                    )
                    nc.sync.dma_start(out=outt_d[:, c0 : c0 + NTD], in_=mt[:])
            dpool.release()
            if dppool is not None:
                dppool.release()

    nc.compile()
    return nc


def _assemble(results, col, n_real, dtype):
    n = sum(n_real)
    out = np.empty((n, D), dtype=dtype)
    for c in range(NCORES):
        nr = n_real[c]
        if nr == 0:
            continue
        outt = results[c]["outt"]  # [D, SHARD] in hist-column order
        out[c * SHARD : c * SHARD + nr] = outt[:, col[:nr]].T
    return out


def kernel(x, W, edge_index, atom_types):
    x = np.asarray(x)
    in_maps, col, n_real = _host_prep(x, W, edge_index, atom_types)
    nc = build_nc()
    res = run_bass_kernel_spmd(nc, in_maps, list(range(NCORES)))
    return _assemble(res.results, col, n_real, x.dtype)



# revision 2
# speedup vs baseline: 9.4670x; 9.4670x over previous
"""Trainium2 Bass kernel v4 for nn_AtomwiseLinear (histogram_binning).

Pure-DVE histogram, transposed layout (per core, SPMD x8, no collectives):
  host: degree-balanced assignment of 4 nodes to each of 32768 columns;
        column cap E_CAP = max balanced load (~128, adaptive, no
        quantization). Each edge endpoint becomes one byte
        z = lo + 4*(1-w) (lo = node slot in column, w = other endpoint's
        type bit), padded 255. Columns live on PARTITIONS (p = col%128),
        entries along the free dim.
  device:
    A) per iteration (512 columns = 4 groups): DMA the byte block,
       ACT-copy u8 -> bf16, one DVE is_equal builds the 8-wide one-hot
       [128, 4*8*E_CAP] (bf16, packed: 2x-mode eligible), one DVE
       tensor_reduce(axis=X, add) sums entries -> counts [128, 32] bf16
       straight into the hist tile. Dense y = x@W (bf16) on PE with ACT
       copies into an SBUF-resident y. No PSUM for the histogram.
    B) decode crit = 3*(count>10) + mix with ~10 DVE ops on [128, 1024].
    C) broadcast crit to the dense layout via a DRAM bounce, mask y
       in-place, DMA out as bf16.
  host: inverse-permute into [1M, 30] float32.
"""

import os
import sys

sys.path.insert(0, "/opt/trn_rl_repo")

import numpy as np
import ml_dtypes

from concourse import bacc, bass, mybir
import concourse.tile as tile
from concourse.bass_utils import run_bass_kernel_spmd

BF16 = ml_dtypes.bfloat16

NCORES = 8
N_NODES = 1_000_000
D = 30
WINDOW = 5
DEG_THRESH = 10

NCOL = 32768              # columns per core (4 nodes each)
NG = NCOL // 128          # column groups (one column per partition)
GB = 4                    # groups per DVE iteration
NIT = NG // GB            # iterations
Q = NCOL // 32            # crit cols ( = NG*4 )
SE = 4 * NCOL             # nodes (incl ghosts) per core

F32 = mybir.dt.float32
BF = mybir.dt.bfloat16
U8 = mybir.dt.uint8
FP8 = mybir.dt.float8e4
FP8_NP = mybir.dt.np(FP8)

KCH = int(os.environ.get("KCH", "8192"))      # mask chunk cols


def _balance(deg, ncol_g):
    """Assign 4 nodes to each of ncol_g global columns, equalizing the
    per-column degree sums (matched greedy per round)."""
    NT = 4 * ncol_g
    d = np.zeros(NT, np.int64)
    d[:deg.shape[0]] = deg
    order = np.argsort(-d, kind="stable")
    loads = np.zeros(ncol_g, np.int64)
    col = np.empty(NT, np.int64)
    slot = np.empty(NT, np.int64)
    for r in range(4):
        seg = order[r * ncol_g:(r + 1) * ncol_g]           # degrees desc
        tgt = np.argsort(-loads, kind="stable")            # loads desc
        col[seg[::-1]] = tgt                               # asc deg -> desc load
        slot[seg] = r
        loads[tgt] += d[seg[::-1]]
    return col, slot, int(loads.max()), NT


def _host_prep(x, W, edge_index, atom_types):
    n = x.shape[0]
    e0 = np.asarray(edge_index[0], dtype=np.int32)
    e1 = np.asarray(edge_index[1], dtype=np.int32)
    t8 = np.asarray(atom_types, dtype=np.uint8)

    deg = np.bincount(e0, minlength=n) + np.bincount(e1, minlength=n)
    col, slot, maxload, NT = _balance(deg, NCORES * NCOL)
    ECAP = max(maxload, 64)
    NCOL_G = NCORES * NCOL

    lo_n = slot.astype(np.uint8)          # node slot within column [0,4)
    core_n = col // NCOL
    cl_n = col % NCOL                     # local column

    # --- entry streams: byte z = lo + 4*(1-w), pad 255 ---
    nodes = np.concatenate([e0, e1])
    wbit = np.concatenate([t8[e1], t8[e0]])
    gc = col[nodes]                       # global column of each entry
    order2 = np.argsort(gc, kind="stable")
    gcs = gc[order2]
    sn = nodes[order2]
    sw = wbit[order2]
    counts = np.bincount(gc, minlength=NCOL_G)
    assert counts.max() <= ECAP, (counts.max(), ECAP)
    starts = np.zeros(NCOL_G, dtype=np.int64)
    np.cumsum(counts[:-1], out=starts[1:])
    within = np.arange(nodes.shape[0], dtype=np.int64) - starts[gcs]

    # stream [core][it][p][gb*ECAP + e], column cl = (it*GB+gb)*128 + p
    stream = np.full(NCORES * NIT * 128 * GB * ECAP, 255, np.uint8)
    c_ = gcs // NCOL
    cll = gcs % NCOL
    g_ = cll // 128
    p_ = cll % 128
    it_ = g_ // GB
    gb_ = g_ % GB
    idx = ((c_ * NIT + it_) * 128 + p_) * (GB * ECAP) + gb_ * ECAP + within
    stream[idx] = lo_n[sn] + 4 * (1 - sw)
    stream = stream.reshape(NCORES, NIT, 128, GB * ECAP)

    # --- node -> hist/dense position ---
    # node (cl, lo): p = cl%128, g = cl//128; crit q = g*4 + lo;
    # hist col = g*8 + z; dense j = p*Q + q
    g_n = cl_n // 128
    p_n = cl_n % 128
    q_n = g_n * 4 + lo_n
    jg = core_n * SE + p_n * Q + q_n
    inv = np.empty(NT, np.int64)
    inv[jg] = np.arange(NT)

    xfull = np.zeros((NT, D), np.float32)
    xfull[:n] = np.asarray(x, np.float32)
    tfull = np.zeros(NT, np.uint8)
    tfull[:n] = t8
    xg = xfull[inv]                           # dense order
    tg = tfull[inv]
    xt = np.ascontiguousarray(
        xg.reshape(NCORES, SE, D).transpose(0, 2, 1)
    ).astype(BF16)
    th = tg.reshape(NCORES, 128, Q).astype(FP8_NP)

    wsc = (np.asarray(W, np.float64) / np.sqrt(D)).astype(np.float32).astype(BF16)
    d5v = np.arange(128, dtype=np.float32) % 32
    d5 = np.where(d5v < 30, d5v // WINDOW, 99.0).reshape(128, 1).astype(np.float32)
    # iota: value z repeated ECAP times, for all 8 z -> [128, 8*ECAP] bf16
    iota = np.repeat(np.arange(8, dtype=np.uint8), ECAP).reshape(1, -1)
    iota = np.broadcast_to(iota, (128, 8 * ECAP)).copy()

    in_maps = []
    for c in range(NCORES):
        in_maps.append({
            "stream": stream[c], "xt": xt[c], "th": th[c],
            "wsc": wsc, "d5": d5, "iota": iota,
        })
    return in_maps, inv, ECAP


def build_nc(shape=128):
    ECAP = shape
    NDG = NCOL // 512 // NIT    # dense chunks (of 512 cols x 4 strips) per iter
    nc = bacc.Bacc("TRN2", target_bir_lowering=False, debug=False,
                   num_devices=NCORES)
    stream_d = nc.dram_tensor("stream", [NIT, 128, GB * ECAP], U8,
                              kind="ExternalInput")
    xt_d = nc.dram_tensor("xt", [D, SE], BF, kind="ExternalInput")
    th_d = nc.dram_tensor("th", [128, Q], FP8, kind="ExternalInput")
    wsc_d = nc.dram_tensor("wsc", [D, D], BF, kind="ExternalInput")
    d5_d = nc.dram_tensor("d5", [128, 1], F32, kind="ExternalInput")
    iota_d = nc.dram_tensor("iota", [128, 8 * ECAP], U8, kind="ExternalInput")
    outt_d = nc.dram_tensor("outt", [4, D, NCOL], BF, kind="ExternalOutput")
    critd = nc.dram_tensor("crit_bounce", [1, SE], BF)
    AL = mybir.AluOpType

    with tile.TileContext(nc) as tc:
        with tc.tile_pool(name="const", bufs=1) as cpool:
            iota = cpool.tile([128, 8 * ECAP], U8)
            wsc = cpool.tile([D, D], BF)
            d5 = cpool.tile([128, 1], F32)
            th = cpool.tile([128, Q], FP8)
            hist = cpool.tile([128, NG * 8], F32)
            y = cpool.tile([128, NCOL], BF)

            nc.sync.dma_start(out=iota[:], in_=iota_d[:])
            nc.sync.dma_start(out=wsc[:], in_=wsc_d[:])
            nc.sync.dma_start(out=d5[:], in_=d5_d[:])
            nc.sync.dma_start(out=th[:], in_=th_d[:])

            wpool = tc.alloc_tile_pool(name="work", bufs=3)
            dpool = tc.alloc_tile_pool(name="dpsum", bufs=2, space="PSUM")

            # ---- Phase A: DVE histogram + interleaved dense ----
            for it in range(NIT):
                raw = wpool.tile([128, GB * ECAP], U8, tag="raw")
                nc.sync.dma_start(out=raw[:], in_=stream_d[bass.ds(it, 1), :, :])
                oh = wpool.tile([128, GB * 8 * ECAP], FP8, tag="oh")
                nc.vector.tensor_tensor(
                    out=oh[:].rearrange("p (b z e) -> p b z e", b=GB, z=8),
                    in0=iota[:].rearrange("p (z e) -> p z e", e=ECAP)
                        .unsqueeze(1).to_broadcast([128, GB, 8, ECAP]),
                    in1=raw[:].rearrange("p (b e) -> p b e", b=GB)
                        .unsqueeze(2).to_broadcast([128, GB, 8, ECAP]),
                    op=AL.is_equal,
                )
                with nc.allow_low_precision(reason="counts <= 128 exact in bf16"):
                    nc.vector.tensor_reduce(
                        out=hist[:, it * GB * 8:(it + 1) * GB * 8],
                        in_=oh[:].rearrange("p (z e) -> p z e", e=ECAP),
                        axis=mybir.AxisListType.X,
                        op=AL.add,
                    )
                # dense chunks for this iteration
                for u in range(NDG):
                    m0 = (it * NDG + u) * 512
                    xtt = wpool.tile([D, 4 * 512], BF, tag="xtt")
                    nc.sync.dma_start(
                        out=xtt[:],
                        in_=xt_d[:].rearrange("d (s m) -> d s m", s=4)[
                            :, :, m0:m0 + 512],
                    )
                    dps = dpool.tile([128, 512], F32, tag="dps")
                    for s in range(4):
                        nc.tensor.matmul(
                            dps[32 * s:32 * s + D, :],
                            lhsT=wsc[:],
                            rhs=xtt[:, s * 512:(s + 1) * 512],
                            start=True, stop=True, tile_position=(0, 32 * s),
                        )
                    nc.scalar.copy(out=y[:, m0:m0 + 512], in_=dps[:])

            wpool.release()
            dpool.release()

            # ---- Phase B: decode crit = 3*(count>10) + mix ----
            h8 = hist[:].rearrange("p (b u) -> p b u", u=8)
            av = h8[:, :, 0:4]      # A  = # type-1 neighbors (w=1 block)
            b0 = h8[:, :, 4:8]      # B0 = # type-0 neighbors
            thf = cpool.tile([128, Q], F32)
            nc.scalar.copy(out=thf[:], in_=th[:])
            cnt = cpool.tile([128, Q], F32)
            ta = cpool.tile([128, Q], F32)
            tb = cpool.tile([128, Q], F32)
            crit = cpool.tile([128, Q], BF)

            def v4(t):
                return t[:].rearrange("p (b u) -> p b u", u=4)

            nc.vector.tensor_tensor(out=v4(cnt), in0=av, in1=b0, op=AL.add)
            nc.vector.tensor_scalar(out=v4(ta), in0=av, scalar1=0.0,
                                    scalar2=None, op0=AL.is_equal)
            nc.vector.tensor_scalar(out=tb[:], in0=thf[:], scalar1=-1.0,
                                    scalar2=1.0, op0=AL.mult, op1=AL.add)
            nc.vector.tensor_tensor(out=ta[:], in0=ta[:], in1=tb[:], op=AL.mult)
            nc.vector.tensor_tensor(out=v4(tb), in0=av, in1=v4(cnt), op=AL.is_equal)
            nc.vector.scalar_tensor_tensor(out=tb[:], in0=tb[:], scalar=2.0,
                                           in1=thf[:], op0=AL.mult, op1=AL.mult)
            nc.vector.tensor_tensor(out=ta[:], in0=ta[:], in1=tb[:], op=AL.add)
            nc.vector.tensor_scalar(out=tb[:], in0=cnt[:], scalar1=0.0,
                                    scalar2=None, op0=AL.is_gt)
            nc.vector.tensor_tensor(out=ta[:], in0=ta[:], in1=tb[:], op=AL.mult)
            nc.vector.tensor_scalar(out=tb[:], in0=cnt[:],
                                    scalar1=float(DEG_THRESH) + 0.5,
                                    scalar2=None, op0=AL.is_gt)
            nc.vector.scalar_tensor_tensor(out=crit[:], in0=tb[:], scalar=3.0,
                                           in1=ta[:], op0=AL.mult, op1=AL.add)
            nc.sync.dma_start(
                out=critd[0:1, :].rearrange("o (p q) -> (o p) q", q=Q),
                in_=crit[:])

            # ---- Phase C: mask y in place, write out ----
            mpool = tc.alloc_tile_pool(name="mask", bufs=2)
            CH = min(KCH, NCOL)
            for c0 in range(0, NCOL, CH):
                critb = mpool.tile([128, CH], BF, tag="critb")
                for s in range(4):
                    nc.sync.dma_start(
                        out=critb[32 * s:32 * s + D, :],
                        in_=critd[0:1, bass.ds(s * NCOL + c0, CH)].to_broadcast(
                            [D, CH]),
                    )
                nc.vector.scalar_tensor_tensor(
                    out=y[:, c0:c0 + CH], in0=critb[:], scalar=d5[:],
                    in1=y[:, c0:c0 + CH], op0=AL.is_equal, op1=AL.mult,
                )
                for s in range(4):
                    nc.sync.dma_start(
                        out=outt_d[bass.ds(s, 1), :, c0:c0 + CH],
                        in_=y[32 * s:32 * s + D, c0:c0 + CH],
                    )
            mpool.release()

    nc.compile()
    return nc


def _assemble(results, inv, dtype):
    # results[c]["outt"]: [4, 30, NCOL] bf16, row-major dense order
    big = np.concatenate(
        [np.asarray(results[c]["outt"]).reshape(4, D, NCOL) for c in range(NCORES)],
        axis=0,
    )
    big = big.transpose(0, 2, 1).reshape(-1, D)
    out = np.empty((N_NODES, D), dtype=dtype)
    sel = inv < N_NODES
    out[inv[sel]] = big[sel].astype(dtype)
    return out


def kernel(x, W, edge_index, atom_types):
    x = np.asarray(x)
    in_maps, inv, ecap = _host_prep(x, W, edge_index, atom_types)
    nc = build_nc(shape=ecap)
    res = run_bass_kernel_spmd(nc, in_maps, list(range(NCORES)))
    return _assemble(res.results, inv, np.float32)
